# revision 1
# baseline (speedup 1.0000x reference)
"""Trainium2 Bass kernel for nn_GAT_27539330301988 (2-layer GAT, N=100k, E=6.4M).

Strategy (8 NeuronCores, SPMD):
  - Host does index-only preprocessing: add self loops, sort edges by
    destination, deal nodes round-robin to 8 cores by in-degree rank,
    build per-node padded edge lists (degree-binned groups of 125 nodes,
    4 groups per superblock, padding points at a sentinel table row whose
    attention logit is -1e9 so exp() underflows to 0).
  - All floating-point math runs on device in three SPMD dispatches:
      A1: node table G1[n] = [x@W1 | x@W1@As | x@W1@Ad]  (nodes sharded)
      A2: layer-1 edge pass: per-edge indirect gather of 64B table rows,
          softmax-weighted aggregation per destination node (softmax max
          subtraction is skipped -- mathematically exact by shift
          invariance, and |e| < ~20 so exp() cannot overflow), + b1,
          output transposed per group for dispatch B.
      B:  batch-norm stats + BN + ELU + W2eff table build (G2), then the
          layer-2 edge pass, + b2 -> final output rows.
  - Host re-assembles/permutes device outputs (bitwise moves only).
"""
import numpy as np
from contextlib import ExitStack

import concourse.bass as bass
import concourse.bacc as bacc
import concourse.tile as tile
from concourse import mybir
from concourse.bass_utils import run_bass_kernel_spmd
from concourse.masks import make_identity

F32 = mybir.dt.float32
I32 = mybir.dt.int32
AX = mybir.AxisListType
OP = mybir.AluOpType
AF = mybir.ActivationFunctionType

N = 100000
E = 6400000
NCORES = 8
IN_CH = 128
P = 125              # nodes per group (partition dim)
GSB = 4              # groups per superblock
NSB = 25             # superblocks per core
NGRP = NSB * GSB     # 100 groups per core
MPC = N // NCORES    # 12500 nodes per core
ROWF = 16            # floats per table row (64B, one HBM burst)
SENT = N             # sentinel table row
TAB = N + 1
NBLK = 12            # groups stacked per partition-block in out1st
NW = -(-NCORES * NGRP // NBLK)   # 67 column chunks of width P
EPS_BN = 1e-5


# ---------------------------------------------------------------- host prep
def _prep(edge_index):
    ei = np.asarray(edge_index).astype(np.int64)
    loop = np.arange(N, dtype=np.int64)
    src = np.concatenate([ei[0], loop])
    dst = np.concatenate([ei[1], loop])
    deg = np.bincount(dst, minlength=N)
    order = np.argsort(-deg, kind="stable")
    pi = np.concatenate([order[k::NCORES] for k in range(NCORES)])
    pos = np.empty(N, np.int64)
    pos[pi] = np.arange(N)
    newdeg = deg[pi]
    D = newdeg.reshape(NCORES, NSB, GSB * P).max(axis=(0, 2)).astype(int)

    eorder = np.argsort(pos[dst], kind="stable")
    ssrc = pos[src[eorder]].astype(np.int32)
    starts = np.concatenate([[0], np.cumsum(newdeg)])

    idx_cores = []
    for k in range(NCORES):
        parts = []
        for s in range(NSB):
            Ds = int(D[s])
            npos = k * MPC + s * GSB * P + np.arange(GSB * P)
            F = np.full((GSB * P, Ds), SENT, np.int32)
            d = newdeg[npos]
            jj = np.arange(Ds)[None, :]
            m = jj < d[:, None]
            sidx = (starts[npos][:, None] + jj)[m]
            F[m] = ssrc[sidx]
            parts.append(
                F.reshape(GSB, P, Ds).transpose(1, 0, 2).reshape(P, GSB * Ds))
        idx_cores.append(np.ascontiguousarray(np.concatenate(parts, axis=1)))

    own_cores = []
    for k in range(NCORES):
        g = np.arange(NGRP)[None, :]
        p = np.arange(P)[:, None]
        own_cores.append(
            np.ascontiguousarray((k * MPC + g * P + p).astype(np.int32)))
    return pi, D, idx_cores, own_cores


# ------------------------------------------------------------- kernel A1
def build_a1():
    nc = bacc.Bacc()
    xtp = nc.dram_tensor("xtp", [IN_CH, MPC], F32, kind="ExternalInput")
    w1 = nc.dram_tensor("w1", [IN_CH, 10], F32, kind="ExternalInput")
    w1t = nc.dram_tensor("w1t", [10, IN_CH], F32, kind="ExternalInput")
    asad1 = nc.dram_tensor("asad1", [10, 4], F32, kind="ExternalInput")
    g1s = nc.dram_tensor("g1s", [MPC, 14], F32, kind="ExternalOutput")

    with tile.TileContext(nc) as tc, ExitStack() as ctx:
        res = ctx.enter_context(tc.tile_pool(name="res", bufs=1))
        sb = ctx.enter_context(tc.tile_pool(name="sb", bufs=3))
        ps = ctx.enter_context(tc.tile_pool(name="ps", bufs=3, space="PSUM"))

        w1eff = res.tile([IN_CH, 14], F32)
        nc.sync.dma_start(out=w1eff[:, 0:10], in_=w1[:])
        w1t_s = res.tile([10, IN_CH], F32)
        nc.sync.dma_start(out=w1t_s[:], in_=w1t[:])
        asad_s = res.tile([10, 4], F32)
        nc.sync.dma_start(out=asad_s[:], in_=asad1[:])
        pw = ps.tile([IN_CH, 4], F32, tag="pw")
        nc.tensor.matmul(pw[:], lhsT=w1t_s[:], rhs=asad_s[:], start=True, stop=True)
        nc.vector.tensor_copy(out=w1eff[:, 10:14], in_=pw[:])

        CH = 500  # nodes per x chunk
        for c in range(MPC // CH):
            xc = sb.tile([IN_CH, CH], F32, tag="xc")
            nc.sync.dma_start(out=xc[:], in_=xtp[:, c * CH:(c + 1) * CH])
            for t in range(CH // P):
                pt = ps.tile([P, 14], F32, tag="pt")
                nc.tensor.matmul(pt[:], lhsT=xc[:, t * P:(t + 1) * P],
                                 rhs=w1eff[:], start=True, stop=True)
                row = sb.tile([P, 14], F32, tag="row")
                nc.vector.tensor_copy(out=row[:], in_=pt[:])
                a = c * CH + t * P
                nc.sync.dma_start(out=g1s[a:a + P, :], in_=row[:])
    nc.compile()
    return nc


# ------------------------------------------------------------- kernel A2
def build_a2(D):
    icols = GSB * int(np.sum(D))
    nc = bacc.Bacc()
    g1 = nc.dram_tensor("g1", [TAB, ROWF], F32, kind="ExternalInput")
    idx = nc.dram_tensor("idx", [P, icols], I32, kind="ExternalInput")
    own = nc.dram_tensor("own", [P, NGRP], I32, kind="ExternalInput")
    b1r = nc.dram_tensor("b1r", [P, 10], F32, kind="ExternalInput")
    out1t = nc.dram_tensor("out1t", [NGRP * 10, P], F32, kind="ExternalOutput")

    with tile.TileContext(nc) as tc, ExitStack() as ctx:
        res = ctx.enter_context(tc.tile_pool(name="res", bufs=1))
        sb = ctx.enter_context(tc.tile_pool(name="sb", bufs=2))
        ps = ctx.enter_context(tc.tile_pool(name="ps", bufs=2, space="PSUM"))

        idxall = res.tile([P, icols], I32)
        nc.sync.dma_start(out=idxall[:], in_=idx[:])
        ownall = res.tile([P, NGRP], I32)
        nc.sync.dma_start(out=ownall[:], in_=own[:])
        b1t = res.tile([P, 10], F32)
        nc.sync.dma_start(out=b1t[:], in_=b1r[:])
        idt = res.tile([P, P], F32)
        make_identity(nc, idt[:])

        coff = 0
        for s in range(NSB):
            Ds = int(D[s])
            g = sb.tile([P, GSB * Ds * ROWF], F32, tag="g")
            for j in range(GSB * Ds):
                nc.gpsimd.indirect_dma_start(
                    out=g[:, j * ROWF:(j + 1) * ROWF], out_offset=None,
                    in_=g1[:],
                    in_offset=bass.IndirectOffsetOnAxis(
                        ap=idxall[:, coff + j:coff + j + 1], axis=0))
            o = sb.tile([P, GSB * ROWF], F32, tag="o")
            for j in range(GSB):
                nc.gpsimd.indirect_dma_start(
                    out=o[:, j * ROWF:(j + 1) * ROWF], out_offset=None,
                    in_=g1[:],
                    in_offset=bass.IndirectOffsetOnAxis(
                        ap=ownall[:, GSB * s + j:GSB * s + j + 1], axis=0))
            coff += GSB * Ds

            g4 = g[:].rearrange("p (g d c) -> p g d c", g=GSB, c=ROWF)
            o3 = o[:].rearrange("p (g c) -> p g c", c=ROWF)
            ex = sb.tile([P, GSB * Ds * 2], F32, tag="ex")
            ex4 = ex[:].rearrange("p (g d h) -> p g d h", g=GSB, h=2)
            nc.vector.tensor_tensor(
                out=ex4[:, :, :, :], in0=g4[:, :, :, 10:12],
                in1=o3[:, :, None, 12:14].broadcast_to([P, GSB, Ds, 2]),
                op=OP.add)
            ext = sb.tile([P, GSB * Ds * 2], F32, tag="ext")
            nc.vector.tensor_scalar(out=ext[:], in0=ex[:], scalar1=0.2,
                                    scalar2=None, op0=OP.mult)
            nc.vector.tensor_tensor(out=ex[:], in0=ex[:], in1=ext[:], op=OP.max)
            nc.scalar.activation(out=ex[:], in_=ex[:], func=AF.Exp)

            msg = sb.tile([P, GSB * Ds * 10], F32, tag="msg")
            msg4 = msg[:].rearrange("p (g d c) -> p g d c", g=GSB, c=10)
            for h in range(2):
                nc.vector.tensor_tensor(
                    out=msg4[:, :, :, 5 * h:5 * h + 5],
                    in0=g4[:, :, :, 5 * h:5 * h + 5],
                    in1=ex4[:, :, :, h:h + 1].broadcast_to([P, GSB, Ds, 5]),
                    op=OP.mult)

            accm = sb.tile([P, GSB * 10], F32, tag="accm")
            nc.vector.tensor_reduce(
                out=accm[:].rearrange("p (g c) -> p g c", g=GSB),
                in_=msg[:].rearrange("p (g d c) -> p g c d", g=GSB, c=10),
                axis=AX.X, op=OP.add)
            acce = sb.tile([P, GSB * 2], F32, tag="acce")
            nc.vector.tensor_reduce(
                out=acce[:].rearrange("p (g h) -> p g h", g=GSB),
                in_=ex[:].rearrange("p (g d h) -> p g h d", g=GSB, h=2),
                axis=AX.X, op=OP.add)
            nc.vector.tensor_scalar(out=acce[:], in0=acce[:], scalar1=1e-16,
                                    scalar2=None, op0=OP.add)
            nc.vector.reciprocal(out=acce[:], in_=acce[:])

            o1 = sb.tile([P, GSB * 10], F32, tag="o1")
            o1v = o1[:].rearrange("p (g h c) -> p g h c", g=GSB, h=2)
            nc.vector.tensor_tensor(
                out=o1v[:, :, :, :],
                in0=accm[:].rearrange("p (g h c) -> p g h c", g=GSB, h=2),
                in1=acce[:].rearrange("p (g h) -> p g h", g=GSB)
                    [:, :, :, None].broadcast_to([P, GSB, 2, 5]),
                op=OP.mult)
            nc.vector.tensor_tensor(
                out=o1[:].rearrange("p (g c) -> p g c", g=GSB),
                in0=o1[:].rearrange("p (g c) -> p g c", g=GSB),
                in1=b1t[:].unsqueeze(1).broadcast_to([P, GSB, 10]),
                op=OP.add)

            pst = ps.tile([GSB * 10, P], F32, tag="pst")
            nc.tensor.transpose(out=pst[:], in_=o1[:], identity=idt[:])
            o1tt = sb.tile([GSB * 10, P], F32, tag="o1tt")
            nc.vector.tensor_copy(out=o1tt[:], in_=pst[:])
            nc.sync.dma_start(out=out1t[s * GSB * 10:(s + 1) * GSB * 10, :],
                              in_=o1tt[:])
    nc.compile()
    return nc


# ------------------------------------------------------------- kernel B
def build_b(D):
    icols = GSB * int(np.sum(D))
    XW = NW * P
    nc = bacc.Bacc()
    x1d = nc.dram_tensor("x1d", [NBLK * 10, XW], F32, kind="ExternalInput")
    idx = nc.dram_tensor("idx", [P, icols], I32, kind="ExternalInput")
    own = nc.dram_tensor("own", [P, NGRP], I32, kind="ExternalInput")
    sel = nc.dram_tensor("sel", [NBLK * 10, 10], F32, kind="ExternalInput")
    selt = nc.dram_tensor("selt", [10, NBLK * 10], F32, kind="ExternalInput")
    w2 = nc.dram_tensor("w2", [10, 12], F32, kind="ExternalInput")
    w2t = nc.dram_tensor("w2t", [10, 10], F32, kind="ExternalInput")
    asad2 = nc.dram_tensor("asad2", [10, 2], F32, kind="ExternalInput")
    gamma = nc.dram_tensor("gamma", [10], F32, kind="ExternalInput")
    beta = nc.dram_tensor("beta", [10], F32, kind="ExternalInput")
    b2r = nc.dram_tensor("b2r", [P, 10], F32, kind="ExternalInput")
    out2 = nc.dram_tensor("out2", [MPC, 10], F32, kind="ExternalOutput")
    g2 = nc.dram_tensor("g2", [TAB, ROWF], F32)

    with tile.TileContext(nc) as tc, ExitStack() as ctx:
        res = ctx.enter_context(tc.tile_pool(name="res", bufs=1))
        sb = ctx.enter_context(tc.tile_pool(name="sb", bufs=2))
        tb = ctx.enter_context(tc.tile_pool(name="tb", bufs=4))
        ps = ctx.enter_context(tc.tile_pool(name="ps", bufs=1, space="PSUM"))
        pst4 = ctx.enter_context(tc.tile_pool(name="pst4", bufs=4, space="PSUM"))

        # resident input activations [120, 8375]
        x1 = res.tile([NBLK * 10, XW], F32)
        nc.sync.dma_start(out=x1[:], in_=x1d[:])

        # ---- BN statistics ----
        stats_pool = tc.tile_pool(name="stats", bufs=1)
        stp = stats_pool.__enter__()
        sel_s = stp.tile([NBLK * 10, 10], F32)
        nc.sync.dma_start(out=sel_s[:], in_=sel[:])
        selt_s = stp.tile([10, NBLK * 10], F32)
        nc.sync.dma_start(out=selt_s[:], in_=selt[:])

        st12 = stp.tile([NBLK * 10, 2], F32, tag="st12")
        nc.vector.tensor_reduce(out=st12[:, 0:1], in_=x1[:], axis=AX.X, op=OP.add)
        sq = stp.tile([NBLK * 10, XW], F32)
        nc.vector.tensor_tensor(out=sq[:], in0=x1[:], in1=x1[:], op=OP.mult)
        nc.vector.tensor_reduce(out=st12[:, 1:2], in_=sq[:], axis=AX.X, op=OP.add)
        pfold = ps.tile([10, 2], F32, tag="pfold")
        nc.tensor.matmul(pfold[:], lhsT=sel_s[:], rhs=st12[:], start=True, stop=True)
        mm = stp.tile([10, 2], F32, tag="mm")
        nc.vector.tensor_scalar(out=mm[:], in0=pfold[:], scalar1=1.0 / N,
                                scalar2=None, op0=OP.mult)
        var = stp.tile([10, 1], F32, tag="var")
        nc.vector.tensor_tensor(out=var[:], in0=mm[:, 0:1], in1=mm[:, 0:1],
                                op=OP.mult)
        nc.vector.tensor_tensor(out=var[:], in0=mm[:, 1:2], in1=var[:],
                                op=OP.subtract)
        nc.vector.tensor_scalar(out=var[:], in0=var[:], scalar1=EPS_BN,
                                scalar2=None, op0=OP.add)
        nc.vector.reciprocal(out=var[:], in_=var[:])
        rstd = stp.tile([10, 1], F32, tag="rstd")
        nc.scalar.activation(out=rstd[:], in_=var[:], func=AF.Sqrt)
        gt = stp.tile([10, 1], F32, tag="gt")
        nc.sync.dma_start(out=gt[:], in_=gamma[:, None])
        bt = stp.tile([10, 1], F32, tag="bt")
        nc.sync.dma_start(out=bt[:], in_=beta[:, None])
        sc2 = stp.tile([10, 2], F32, tag="sc2")
        nc.vector.tensor_tensor(out=sc2[:, 0:1], in0=rstd[:], in1=gt[:], op=OP.mult)
        nc.vector.tensor_tensor(out=sc2[:, 1:2], in0=mm[:, 0:1], in1=sc2[:, 0:1],
                                op=OP.mult)
        nc.vector.tensor_tensor(out=sc2[:, 1:2], in0=bt[:], in1=sc2[:, 1:2],
                                op=OP.subtract)
        prep = ps.tile([NBLK * 10, 2], F32, tag="prep")
        nc.tensor.matmul(prep[:], lhsT=selt_s[:], rhs=sc2[:], start=True, stop=True)
        ssr = stp.tile([NBLK * 10, 2], F32, tag="ssr")
        nc.vector.tensor_copy(out=ssr[:], in_=prep[:])

        # ---- BN + ELU in place ----
        nc.vector.tensor_scalar(out=x1[:], in0=x1[:], scalar1=ssr[:, 0:1],
                                scalar2=ssr[:, 1:2], op0=OP.mult, op1=OP.add)
        nc.vector.tensor_scalar(out=sq[:], in0=x1[:], scalar1=0.0,
                                scalar2=None, op0=OP.min)
        nc.scalar.activation(out=sq[:], in_=sq[:], func=AF.Exp)
        nc.vector.tensor_scalar(out=sq[:], in0=sq[:], scalar1=-1.0,
                                scalar2=None, op0=OP.add)
        nc.vector.tensor_tensor(out=x1[:], in0=x1[:], in1=sq[:], op=OP.max)
        stats_pool.__exit__(None, None, None)

        # ---- W2eff ----
        w2eff = res.tile([10, 12], F32)
        nc.sync.dma_start(out=w2eff[:, 0:10], in_=w2[:, 0:10])
        w2t_s = sb.tile([10, 10], F32, tag="w2ts")
        nc.sync.dma_start(out=w2t_s[:], in_=w2t[:])
        asad2_s = sb.tile([10, 2], F32, tag="asad2")
        nc.sync.dma_start(out=asad2_s[:], in_=asad2[:])
        pw2 = ps.tile([10, 2], F32, tag="pw2")
        nc.tensor.matmul(pw2[:], lhsT=w2t_s[:], rhs=asad2_s[:], start=True, stop=True)
        nc.vector.tensor_copy(out=w2eff[:, 10:12], in_=pw2[:])

        # ---- sentinel row ----
        sent = sb.tile([1, ROWF], F32, tag="sent")
        nc.gpsimd.memset(sent[:], 0.0)
        nc.gpsimd.memset(sent[0:1, 10:11], -1e9)
        nc.sync.dma_start(out=g2[SENT:SENT + 1, :], in_=sent[:])

        # ---- G2 table build ----
        # SBUF AP partition bases must be 0/32/64, so restage each 10-row
        # block of the BN'd activations at partition 0 via a DRAM round trip.
        x1bn = nc.dram_tensor("x1bn", [NBLK * 10, XW], F32)
        nc.sync.dma_start(out=x1bn[:, :], in_=x1[:])
        tc.strict_bb_all_engine_barrier()
        stage_pool = tc.tile_pool(name="stage", bufs=1)
        sgp = stage_pool.__enter__()
        for b in range(NBLK):
            stage = sgp.tile([10, XW], F32, tag="stage")
            nc.sync.dma_start(out=stage[:], in_=x1bn[b * 10:(b + 1) * 10, :])
            for w in range(NW):
                grp = w * NBLK + b
                if grp >= NCORES * NGRP:
                    continue
                pt = pst4.tile([P, 12], F32, tag="pt")
                nc.tensor.matmul(pt[:],
                                 lhsT=stage[:, w * P:(w + 1) * P],
                                 rhs=w2eff[:], start=True, stop=True)
                rt = tb.tile([P, 12], F32, tag="rt")
                nc.vector.tensor_copy(out=rt[:], in_=pt[:])
                nc.sync.dma_start(out=g2[grp * P:(grp + 1) * P, 0:12], in_=rt[:])

        stage_pool.__exit__(None, None, None)
        tc.strict_bb_all_engine_barrier()

        # ---- layer-2 edge pass ----
        idxall = res.tile([P, icols], I32)
        nc.sync.dma_start(out=idxall[:], in_=idx[:])
        ownall = res.tile([P, NGRP], I32)
        nc.sync.dma_start(out=ownall[:], in_=own[:])
        b2t = res.tile([P, 10], F32)
        nc.sync.dma_start(out=b2t[:], in_=b2r[:])

        coff = 0
        for s in range(NSB):
            Ds = int(D[s])
            g = sb.tile([P, GSB * Ds * ROWF], F32, tag="g")
            for j in range(GSB * Ds):
                nc.gpsimd.indirect_dma_start(
                    out=g[:, j * ROWF:(j + 1) * ROWF], out_offset=None,
                    in_=g2[:],
                    in_offset=bass.IndirectOffsetOnAxis(
                        ap=idxall[:, coff + j:coff + j + 1], axis=0))
            o = sb.tile([P, GSB * ROWF], F32, tag="o")
            for j in range(GSB):
                nc.gpsimd.indirect_dma_start(
                    out=o[:, j * ROWF:(j + 1) * ROWF], out_offset=None,
                    in_=g2[:],
                    in_offset=bass.IndirectOffsetOnAxis(
                        ap=ownall[:, GSB * s + j:GSB * s + j + 1], axis=0))
            coff += GSB * Ds

            g4 = g[:].rearrange("p (g d c) -> p g d c", g=GSB, c=ROWF)
            o3 = o[:].rearrange("p (g c) -> p g c", c=ROWF)
            ex = sb.tile([P, GSB * Ds], F32, tag="ex")
            ex3 = ex[:].rearrange("p (g d) -> p g d", g=GSB)
            nc.vector.tensor_tensor(
                out=ex3[:, :, :], in0=g4[:, :, :, 10],
                in1=o3[:, :, 11:12].broadcast_to([P, GSB, Ds]),
                op=OP.add)
            ext = sb.tile([P, GSB * Ds], F32, tag="ext")
            nc.vector.tensor_scalar(out=ext[:], in0=ex[:], scalar1=0.2,
                                    scalar2=None, op0=OP.mult)
            nc.vector.tensor_tensor(out=ex[:], in0=ex[:], in1=ext[:], op=OP.max)
            nc.scalar.activation(out=ex[:], in_=ex[:], func=AF.Exp)

            msg = sb.tile([P, GSB * Ds * 10], F32, tag="msg")
            msg4 = msg[:].rearrange("p (g d c) -> p g d c", g=GSB, c=10)
            nc.vector.tensor_tensor(
                out=msg4[:, :, :, :],
                in0=g4[:, :, :, 0:10],
                in1=ex3[:, :, :, None].broadcast_to([P, GSB, Ds, 10]),
                op=OP.mult)

            accm = sb.tile([P, GSB * 10], F32, tag="accm")
            nc.vector.tensor_reduce(
                out=accm[:].rearrange("p (g c) -> p g c", g=GSB),
                in_=msg[:].rearrange("p (g d c) -> p g c d", g=GSB, c=10),
                axis=AX.X, op=OP.add)
            acce = sb.tile([P, GSB], F32, tag="acce")
            nc.vector.tensor_reduce(
                out=acce[:],
                in_=ex[:].rearrange("p (g d) -> p g d", g=GSB),
                axis=AX.X, op=OP.add)
            nc.vector.tensor_scalar(out=acce[:], in0=acce[:], scalar1=1e-16,
                                    scalar2=None, op0=OP.add)
            nc.vector.reciprocal(out=acce[:], in_=acce[:])

            o2 = sb.tile([P, GSB * 10], F32, tag="o2")
            o2v = o2[:].rearrange("p (g c) -> p g c", g=GSB)
            nc.vector.tensor_tensor(
                out=o2v[:, :, :],
                in0=accm[:].rearrange("p (g c) -> p g c", g=GSB),
                in1=acce[:].unsqueeze(2).broadcast_to([P, GSB, 10]),
                op=OP.mult)
            nc.vector.tensor_tensor(
                out=o2v[:, :, :], in0=o2v[:, :, :],
                in1=b2t[:].unsqueeze(1).broadcast_to([P, GSB, 10]),
                op=OP.add)
            nc.sync.dma_start(
                out=out2[s * GSB * P:(s + 1) * GSB * P, :].rearrange(
                    "(g p) c -> p g c", p=P),
                in_=o2v[:, :, :])
    nc.compile()
    return nc


# ---------------------------------------------------------------- driver
def kernel(x, W1, a_src1, a_dst1, b1, gamma1, beta1, W2, a_src2, a_dst2, b2,
           edge_index):
    x = np.ascontiguousarray(np.asarray(x, dtype=np.float32))
    W1 = np.asarray(W1, np.float32)
    W2 = np.asarray(W2, np.float32)
    a_src1 = np.asarray(a_src1, np.float32)
    a_dst1 = np.asarray(a_dst1, np.float32)
    a_src2 = np.asarray(a_src2, np.float32)
    a_dst2 = np.asarray(a_dst2, np.float32)
    b1 = np.asarray(b1, np.float32)
    b2 = np.asarray(b2, np.float32)
    gamma1 = np.asarray(gamma1, np.float32)
    beta1 = np.asarray(beta1, np.float32)

    pi, D, idx_cores, own_cores = _prep(edge_index)
    cores = list(range(NCORES))

    # ---- A1: node table shards ----
    xt = np.ascontiguousarray(x.T)          # [128, N]
    asad1 = np.zeros((10, 4), np.float32)   # [As | Ad] block-diagonal layout
    for h in range(2):
        asad1[5 * h:5 * h + 5, h] = a_src1[h]
        asad1[5 * h:5 * h + 5, 2 + h] = a_dst1[h]
    w1t = np.ascontiguousarray(W1.T)
    in_maps = []
    for k in cores:
        in_maps.append({
            "xtp": np.ascontiguousarray(xt[:, pi[k * MPC:(k + 1) * MPC]]),
            "w1": W1, "w1t": w1t, "asad1": asad1,
        })
    nc1 = build_a1()
    r1 = run_bass_kernel_spmd(nc1, in_maps, cores)

    g1 = np.zeros((TAB, ROWF), np.float32)
    for k in cores:
        g1[k * MPC:(k + 1) * MPC, 0:14] = r1.results[k]["g1s"]
    g1[SENT, 10:12] = -1e9

    # ---- A2: layer-1 edge pass ----
    b1r = np.ascontiguousarray(np.tile(b1, (P, 1)))
    in_maps = []
    for k in cores:
        in_maps.append({
            "g1": g1, "idx": idx_cores[k], "own": own_cores[k], "b1r": b1r,
        })
    nc2 = build_a2(D)
    r2 = run_bass_kernel_spmd(nc2, in_maps, cores)

    # assemble stacked transposed activations [120, NW*125]
    x1 = np.zeros((NBLK * 10, NW * P), np.float32)
    for k in cores:
        o1t = r2.results[k]["out1t"]        # [1000, 125]
        for gi in range(NGRP):
            s, gg = divmod(gi, GSB)
            grp = k * NGRP + gi
            w, b = divmod(grp, NBLK)
            x1[b * 10:(b + 1) * 10, w * P:(w + 1) * P] = \
                o1t[s * GSB * 10 + gg * 10: s * GSB * 10 + gg * 10 + 10, :]

    # ---- B: BN + ELU + table + layer-2 edge pass ----
    sel = np.zeros((NBLK * 10, 10), np.float32)
    sel[np.arange(NBLK * 10), np.arange(NBLK * 10) % 10] = 1.0
    selt = np.ascontiguousarray(sel.T)
    w2in = np.zeros((10, 12), np.float32)
    w2in[:, 0:10] = W2
    asad2 = np.zeros((10, 2), np.float32)
    asad2[:, 0] = a_src2[0]
    asad2[:, 1] = a_dst2[0]
    w2t = np.ascontiguousarray(W2.T)
    b2r = np.ascontiguousarray(np.tile(b2, (P, 1)))
    in_maps = []
    for k in cores:
        in_maps.append({
            "x1d": x1, "idx": idx_cores[k], "own": own_cores[k],
            "sel": sel, "selt": selt, "w2": w2in, "w2t": w2t, "asad2": asad2,
            "gamma": gamma1, "beta": beta1, "b2r": b2r,
        })
    nc3 = build_b(D)
    r3 = run_bass_kernel_spmd(nc3, in_maps, cores)

    out = np.empty((N, 10), np.float32)
    shards = np.concatenate([r3.results[k]["out2"] for k in cores], axis=0)
    out[pi] = shards
    return out

